# revision 5
# baseline (speedup 1.0000x reference)
import os
import sys
from contextlib import ExitStack

import numpy as np

sys.path.insert(0, "/opt/trn_rl_repo")

import concourse.bacc as bacc
import concourse.tile as tile
from concourse import bass, mybir
from concourse.bass_utils import run_bass_kernel_spmd

N_SEND = 49152
N_REC = 65536
N_EDGES = 262144
D_MODEL = 256
EDGE_IN, EDGE_OUT = 4, 64
LIN_IN, LIN_OUT = 320, 256
B = 2

N_CORES = 8
P = 128
N_TILES = N_REC // P
T_PER_CORE = N_TILES // N_CORES

F32 = mybir.dt.float32
I32 = mybir.dt.int32
AF = mybir.ActivationFunctionType
ALU = mybir.AluOpType

TRACE = bool(os.environ.get("KERNEL_TRACE", ""))
LAST_EXEC_NS = None
LAST_TRACE = None
LAST_PROFILE_JSON = None

_PROGRAM_CACHE = {}


def _build_program(NB: int, n_tiles: int = T_PER_CORE):
    nc = bacc.Bacc("TRN2", target_bir_lowering=False, debug=False)
    cap = NB * P

    xcat = nc.dram_tensor("xcat", [N_SEND, 2 * D_MODEL], F32, kind="ExternalInput").ap()
    sid = nc.dram_tensor("sid", [P, T_PER_CORE * NB], I32, kind="ExternalInput").ap()
    recl = nc.dram_tensor("recl", [P, T_PER_CORE * NB], F32, kind="ExternalInput").ap()
    attrT = nc.dram_tensor(
        "attrT", [T_PER_CORE, EDGE_IN + 1, cap], F32, kind="ExternalInput"
    ).ap()
    w1a = nc.dram_tensor("w1a", [EDGE_IN + 1, EDGE_OUT], F32, kind="ExternalInput").ap()
    w2a = nc.dram_tensor("w2a", [EDGE_OUT + 1, EDGE_OUT], F32, kind="ExternalInput").ap()
    ffw1 = nc.dram_tensor("ffw1", [LIN_IN, LIN_OUT], F32, kind="ExternalInput").ap()
    ffw2 = nc.dram_tensor("ffw2", [LIN_OUT, LIN_OUT], F32, kind="ExternalInput").ap()
    b1c = nc.dram_tensor("b1c", [P, 2], F32, kind="ExternalInput").ap()
    b2bc = nc.dram_tensor("b2bc", [P, 2 * LIN_OUT], F32, kind="ExternalInput").ap()
    iota = nc.dram_tensor("iota", [P, P], F32, kind="ExternalInput").ap()
    out = nc.dram_tensor(
        "out", [B, T_PER_CORE * P, LIN_OUT], F32, kind="ExternalOutput"
    ).ap()

    with tile.TileContext(nc) as tc:
        with ExitStack() as ctx:
            const = ctx.enter_context(tc.tile_pool(name="const", bufs=1))
            apool = ctx.enter_context(tc.tile_pool(name="attr", bufs=2))
            hpool = ctx.enter_context(tc.tile_pool(name="hta", bufs=2))
            gpool = ctx.enter_context(tc.tile_pool(name="gather", bufs=NB + 3))
            opool = ctx.enter_context(tc.tile_pool(name="onehot", bufs=NB + 3))
            epool = ctx.enter_context(tc.tile_pool(name="ef", bufs=NB + 3))
            vpool = ctx.enter_context(tc.tile_pool(name="vsb", bufs=2))
            h2pool = ctx.enter_context(tc.tile_pool(name="h2", bufs=2))
            ypool = ctx.enter_context(tc.tile_pool(name="y", bufs=2))
            pacc = ctx.enter_context(tc.tile_pool(name="pacc", bufs=1, space="PSUM"))
            pacc2 = ctx.enter_context(tc.tile_pool(name="pacc2", bufs=1, space="PSUM"))
            pblk = ctx.enter_context(tc.tile_pool(name="pblk", bufs=2, space="PSUM"))
            pblk2 = ctx.enter_context(tc.tile_pool(name="pblk2", bufs=2, space="PSUM"))
            pff = ctx.enter_context(tc.tile_pool(name="pff", bufs=1, space="PSUM"))
            pff2 = ctx.enter_context(tc.tile_pool(name="pff2", bufs=1, space="PSUM"))

            iota_sb = const.tile([P, P], F32)
            nc.sync.dma_start(iota_sb[:], iota[:])
            w1a_sb = const.tile([EDGE_IN + 1, EDGE_OUT], F32)
            nc.sync.dma_start(w1a_sb[:], w1a[:])
            w2a_sb = const.tile([EDGE_OUT + 1, EDGE_OUT], F32)
            nc.sync.dma_start(w2a_sb[:], w2a[:])
            ffw1a_sb = const.tile([P, LIN_OUT], F32)
            nc.sync.dma_start(ffw1a_sb[:], ffw1[0:128, :])
            ffw1b_sb = const.tile([P, LIN_OUT], F32)
            nc.sync.dma_start(ffw1b_sb[:], ffw1[128:256, :])
            ffw1e_sb = const.tile([EDGE_OUT, LIN_OUT], F32)
            nc.sync.dma_start(ffw1e_sb[:], ffw1[256:320, :])
            ffw2_sb = const.tile([P, 2 * LIN_OUT], F32)
            nc.sync.dma_start(ffw2_sb[:, 0:LIN_OUT], ffw2[0:128, :])
            nc.sync.dma_start(ffw2_sb[:, LIN_OUT:], ffw2[128:256, :])
            b1c_sb = const.tile([P, 2], F32)
            nc.sync.dma_start(b1c_sb[:], b1c[:])
            b2bc_sb = const.tile([P, 2 * LIN_OUT], F32)
            nc.sync.dma_start(b2bc_sb[:], b2bc[:])
            sid_sb = const.tile([P, T_PER_CORE * NB], I32)
            nc.sync.dma_start(sid_sb[:], sid[:])
            recl_sb = const.tile([P, T_PER_CORE * NB], F32)
            nc.sync.dma_start(recl_sb[:], recl[:])

            for t in range(n_tiles):
                attr_t = apool.tile([EDGE_IN + 1, cap], F32)
                nc.sync.dma_start(attr_t[:], attrT[t])
                hTa = hpool.tile([EDGE_OUT + 1, cap], F32)
                nc.vector.memset(hTa[EDGE_OUT : EDGE_OUT + 1, :], 1.0)

                ps_vx = pacc.tile([P, 512], F32)
                ps_vef = pacc2.tile([EDGE_OUT, P], F32)

                xgs, ohs, efs = [], [], []
                for nb in range(NB):
                    col = t * NB + nb

                    xg = gpool.tile([P, 2 * D_MODEL], F32)
                    nc.gpsimd.indirect_dma_start(
                        out=xg[:],
                        out_offset=None,
                        in_=xcat,
                        in_offset=bass.IndirectOffsetOnAxis(
                            ap=sid_sb[:, col : col + 1], axis=0
                        ),
                    )
                    xgs.append(xg)

                    oh = opool.tile([P, P], F32)
                    nc.vector.tensor_tensor(
                        oh[:],
                        recl_sb[:, col : col + 1].to_broadcast([P, P]),
                        iota_sb[:],
                        op=ALU.is_equal,
                    )
                    ohs.append(oh)

                    ps_h = pblk.tile([EDGE_OUT, P], F32)
                    nc.tensor.matmul(
                        ps_h[:],
                        w1a_sb[:],
                        attr_t[:, nb * P : (nb + 1) * P],
                        start=True,
                        stop=True,
                    )
                    nc.scalar.activation(
                        hTa[0:EDGE_OUT, nb * P : (nb + 1) * P], ps_h[:], AF.Relu
                    )
                    ps_ef = pblk2.tile([P, EDGE_OUT], F32)
                    nc.tensor.matmul(
                        ps_ef[:],
                        hTa[:, nb * P : (nb + 1) * P],
                        w2a_sb[:],
                        start=True,
                        stop=True,
                    )
                    ef = epool.tile([P, EDGE_OUT], F32)
                    nc.scalar.activation(ef[:], ps_ef[:], AF.Copy)
                    efs.append(ef)

                for c in range(4):
                    for nb in range(NB):
                        nc.tensor.matmul(
                            ps_vx[:, c * P : (c + 1) * P],
                            xgs[nb][:, c * P : (c + 1) * P],
                            ohs[nb][:],
                            start=nb == 0,
                            stop=nb == NB - 1,
                        )
                for nb in range(NB):
                    nc.tensor.matmul(
                        ps_vef[:], efs[nb][:], ohs[nb][:],
                        start=nb == 0, stop=nb == NB - 1,
                    )

                vx = vpool.tile([P, 512], F32)
                nc.vector.tensor_copy(vx[:], ps_vx[:])
                vef = vpool.tile([EDGE_OUT, P], F32)
                nc.vector.tensor_copy(vef[:], ps_vef[:])

                ps_h2 = pff.tile([P, 512], F32)
                for b in range(2):
                    for jc in range(2):
                        g = (b * 2 + jc) * P
                        js = slice(jc * P, (jc + 1) * P)
                        nc.tensor.matmul(
                            ps_h2[:, g : g + P],
                            ffw1a_sb[:, js],
                            vx[:, (b * 2) * P : (b * 2 + 1) * P],
                            start=True,
                            stop=False,
                        )
                        nc.tensor.matmul(
                            ps_h2[:, g : g + P],
                            ffw1b_sb[:, js],
                            vx[:, (b * 2 + 1) * P : (b * 2 + 2) * P],
                            start=False,
                            stop=False,
                        )
                        nc.tensor.matmul(
                            ps_h2[:, g : g + P],
                            ffw1e_sb[:, js],
                            vef[:],
                            start=False,
                            stop=True,
                        )
                h2 = h2pool.tile([P, 512], F32)
                for b in range(2):
                    for jc in range(2):
                        g = (b * 2 + jc) * P
                        nc.scalar.activation(
                            h2[:, g : g + P],
                            ps_h2[:, g : g + P],
                            AF.Relu,
                            bias=b1c_sb[:, jc : jc + 1],
                        )

                ps_y = pff2.tile([P, 2 * LIN_OUT], F32)
                for b in range(2):
                    for kc in range(2):
                        nc.tensor.matmul(
                            ps_y[:, b * LIN_OUT : (b + 1) * LIN_OUT],
                            h2[:, (b * 2 + kc) * P : (b * 2 + kc + 1) * P],
                            ffw2_sb[:, kc * LIN_OUT : (kc + 1) * LIN_OUT],
                            start=kc == 0,
                            stop=kc == 1,
                        )
                y = ypool.tile([P, 2 * LIN_OUT], F32)
                nc.vector.tensor_tensor(y[:], ps_y[:], b2bc_sb[:], op=ALU.add)
                nc.sync.dma_start(out[0, t * P : (t + 1) * P, :], y[:, 0:LIN_OUT])
                nc.sync.dma_start(out[1, t * P : (t + 1) * P, :], y[:, LIN_OUT:])

    nc.compile()
    return nc


def _prepare_inputs(x, edge_index, edge_attr, ee_w1, ee_b1, ee_w2, ee_b2,
                    ff_w1, ff_b1, ff_w2, ff_b2):
    x = np.ascontiguousarray(np.asarray(x, dtype=np.float32))
    edge_index = np.asarray(edge_index, dtype=np.int32)
    edge_attr = np.asarray(edge_attr, dtype=np.float32)
    senders, receivers = edge_index[0], edge_index[1]

    xcat = np.ascontiguousarray(
        np.concatenate([x[0], x[1]], axis=1), dtype=np.float32
    )

    tile_id = (receivers // P).astype(np.int64)
    order = np.argsort(tile_id, kind="stable")
    counts = np.bincount(tile_id, minlength=N_TILES)
    NB = max(1, int(np.ceil(counts.max() / P)))
    cap = NB * P

    offs = np.zeros(N_TILES + 1, np.int64)
    np.cumsum(counts, out=offs[1:])
    tid_sorted = tile_id[order]
    pos = np.arange(N_EDGES, dtype=np.int64) - offs[tid_sorted]

    sid_pad = np.zeros((N_TILES, cap), np.int32)
    recl_pad = np.full((N_TILES, cap), 255.0, np.float32)
    attr_pad = np.zeros((N_TILES, EDGE_IN + 1, cap), np.float32)
    attr_pad[:, EDGE_IN, :] = 1.0
    sid_pad[tid_sorted, pos] = senders[order]
    recl_pad[tid_sorted, pos] = (receivers[order] % P).astype(np.float32)
    ea_sorted = edge_attr[order]
    for c in range(EDGE_IN):
        attr_pad[tid_sorted, c, pos] = ea_sorted[:, c]

    w1a = np.ascontiguousarray(
        np.vstack([np.asarray(ee_w1, np.float32),
                   np.asarray(ee_b1, np.float32)[None, :]]))
    w2a = np.ascontiguousarray(
        np.vstack([np.asarray(ee_w2, np.float32),
                   np.asarray(ee_b2, np.float32)[None, :]]))
    ffw1 = np.ascontiguousarray(np.asarray(ff_w1, np.float32))
    ffw2 = np.ascontiguousarray(np.asarray(ff_w2, np.float32))
    b1c = np.ascontiguousarray(
        np.asarray(ff_b1, np.float32).reshape(2, P).T)
    b2bc = np.ascontiguousarray(
        np.tile(np.asarray(ff_b2, np.float32)[None, :], (P, 2)))
    iota = np.tile(np.arange(P, dtype=np.float32)[None, :], (P, 1))

    in_maps = []
    for c in range(N_CORES):
        tsl = slice(c * T_PER_CORE, (c + 1) * T_PER_CORE)
        sid_c = np.ascontiguousarray(
            sid_pad[tsl].reshape(T_PER_CORE, NB, P).transpose(2, 0, 1)
            .reshape(P, T_PER_CORE * NB))
        recl_c = np.ascontiguousarray(
            recl_pad[tsl].reshape(T_PER_CORE, NB, P).transpose(2, 0, 1)
            .reshape(P, T_PER_CORE * NB))
        in_maps.append({
            "xcat": xcat,
            "sid": sid_c,
            "recl": recl_c,
            "attrT": np.ascontiguousarray(attr_pad[tsl]),
            "w1a": w1a,
            "w2a": w2a,
            "ffw1": ffw1,
            "ffw2": ffw2,
            "b1c": b1c,
            "b2bc": b2bc,
            "iota": iota,
        })
    return NB, in_maps


def _ensure_ntff_hook():
    import types

    import concourse.bass_utils as bu

    bu.upload_artifacts = lambda tmpdir: "local://" + tmpdir
    if "antenv.axon_hooks" in sys.modules:
        return
    import antenv

    mod = types.ModuleType("antenv.axon_hooks")
    mod._hook = None
    mod.set_axon_ntff_profile_hook = lambda h: setattr(mod, "_hook", h)
    mod.get_axon_ntff_profile_hook = lambda: mod._hook
    sys.modules["antenv.axon_hooks"] = mod
    antenv.axon_hooks = mod
    try:
        from trn_agent_boot.trn_boot import _ntff_profile_via_ctypes

        hook = _ntff_profile_via_ctypes("/opt/axon/libaxon_pjrt.so")
        if hook is not None:
            mod._hook = hook
    except Exception as e:
        print(f"ntff hook setup failed: {e}", file=sys.stderr)


def kernel(**inputs) -> np.ndarray:
    global LAST_EXEC_NS, LAST_TRACE, LAST_PROFILE_JSON
    NB, in_maps = _prepare_inputs(**inputs)
    if NB not in _PROGRAM_CACHE:
        _PROGRAM_CACHE[NB] = _build_program(NB)
    nc = _PROGRAM_CACHE[NB]

    kwargs = {}
    if TRACE:
        _ensure_ntff_hook()
        tmpdir = os.environ.get("KERNEL_TRACE_DIR")
        if tmpdir:
            os.makedirs(tmpdir, exist_ok=True)
            kwargs["tmpdir"] = tmpdir
    res = run_bass_kernel_spmd(
        nc, in_maps, list(range(N_CORES)), trace=TRACE, **kwargs
    )
    LAST_EXEC_NS = res.exec_time_ns
    LAST_TRACE = res.instructions_and_trace
    LAST_PROFILE_JSON = res.profile_json
    full = np.empty((B, N_REC, LIN_OUT), np.float32)
    for c in range(N_CORES):
        full[:, c * (N_REC // N_CORES) : (c + 1) * (N_REC // N_CORES), :] = (
            res.results[c]["out"]
        )
    return full
